# revision 6
# baseline (speedup 1.0000x reference)
"""DeepGCN (3x GraphConv+PairNorm+ReLU+residual, then out-spmm) on 8 Trainium2 cores.

Strategy (dst-sharded, per the sharding hint):
  - Nodes are sharded by row across 8 cores (6250 each); edges are partitioned
    by destination so segment_sum stays local to a core.
  - Each layer: dense matmul h = x_loc @ W + b on the local shard (bf16, PE),
    AllGather of h (2 feature chunks, bf16) so every core holds the full gather
    table, then spmm = dma_gather of source rows + PE matmuls against
    precomputed sparse selection matrices S[128 edges, 64 dst slots]
    accumulating in PSUM (segment-sum runs at PE rate).
  - PairNorm stats (column sums + sum of squares) via ones-vector matmuls,
    combined with a tiny AllReduce; normalize/ReLU/residual applied locally.
  - dma_gather indices are int16, so the 50000-row table is addressed as two
    25000-row halves; edge blocks are homogeneous in (dst-tile, window, half).
  - SPMD: one program for all cores, so per-group block counts are padded to
    the max across cores.
"""

import math
import numpy as np
import ml_dtypes

BF16 = ml_dtypes.bfloat16

CFG = dict(
    N=50000,          # nodes
    D=512,            # feature dim
    DOUT=256,         # output classes
    C=8,              # cores
    EPS=1e-6,
    GCH_BLOCKS=16,    # gather-chunk size in 128-edge blocks
)


def _derived(cfg):
    d = dict(cfg)
    d["NSH"] = cfg["N"] // cfg["C"]                # nodes per shard
    d["HALF"] = cfg["N"] // 2                      # gather-table half split
    d["NT"] = math.ceil(d["NSH"] / 128)            # dst tiles per core
    d["NTPAD"] = d["NT"] * 128
    d["WIN"] = 64                                  # dst window (matmul M)
    d["NWIN"] = 128 // d["WIN"]                    # windows per 128-dst tile
    d["KT"] = cfg["D"] // 128                      # k-tiles for dense matmul
    d["FH"] = 2                                    # feature chunks for D tables
    return d


# ---------------- preprocessing -------------------------------------------------

def preprocess(edge_src, edge_dst, edge_w, cfg):
    c = _derived(cfg)
    N, C, NSH, HALF = c["N"], c["C"], c["NSH"], c["HALF"]
    NT, WIN, NWIN = c["NT"], c["WIN"], c["NWIN"]
    CPB = cfg["GCH_BLOCKS"]
    NG = NT * NWIN * 2   # groups per core: (tile, win, half)

    edge_src = edge_src.astype(np.int64)
    edge_dst = edge_dst.astype(np.int64)

    counts = np.zeros((C, NG), np.int64)
    per_core = []
    for cc in range(C):
        base = cc * NSH
        m = (edge_dst >= base) & (edge_dst < base + NSH)
        src = edge_src[m]
        dstl = edge_dst[m] - base
        w = edge_w[m]
        tile = dstl >> 7
        win = (dstl & 127) // WIN
        half = (src >= HALF).astype(np.int64)
        gid = (tile * NWIN + win) * 2 + half
        order = np.lexsort((src, gid))
        src, dstl, w, gid = src[order], dstl[order], w[order], gid[order]
        counts[cc] = np.bincount(gid, minlength=NG)
        per_core.append((src, dstl, w, gid))

    # uniform padded block counts per group (>=1 so every PSUM window is written)
    gmax = counts.max(axis=0)
    nblocks_g = np.maximum((gmax + 127) // 128, 1)
    halves = np.arange(NG) & 1
    bstart_g = np.zeros(NG, np.int64)
    acc = [0, 0]
    for g in range(NG):
        h = halves[g]
        bstart_g[g] = acc[h]
        acc[h] += nblocks_g[g]
    nblk = [acc[0], acc[1]]
    nchunks = [(nblk[h] + CPB - 1) // CPB for h in range(2)]
    nblk_pad = [nchunks[h] * CPB for h in range(2)]

    assert nblocks_g.max() <= CPB, (nblocks_g.max(), CPB)
    meta = dict(cfg=c, nblocks_g=nblocks_g, bstart_g=bstart_g, halves=halves,
                nblk=nblk, nchunks=nchunks, nblk_pad=nblk_pad)

    core_data = []
    for cc in range(C):
        src, dstl, w, gid = per_core[cc]
        gstart = np.zeros(NG, np.int64)
        gstart[1:] = np.cumsum(counts[cc])[:-1]
        rank = np.arange(len(src)) - gstart[gid]
        pos = bstart_g[gid] * 128 + rank          # stream position in edges
        half = halves[gid]

        idx_arrs, s_arrs = [], []
        for h in range(2):
            L = nblk_pad[h] * 128
            idx = np.zeros(L, np.int64)
            sval = np.zeros((L, WIN), np.float32)
            sel = half == h
            p = pos[sel]
            idx[p] = src[sel] - (HALF if h == 1 else 0)
            slot = (dstl[sel] % 128) % WIN
            sval[p, slot] = w[sel]
            assert idx.max(initial=0) < 32768
            idx16 = idx.astype(np.int16).reshape(-1, 16).T        # [16, L/16]
            idx_sb = np.ascontiguousarray(np.tile(idx16, (8, 1)))  # [128, L/16]
            s_b = sval.astype(BF16).reshape(nblk_pad[h], 128, WIN)
            s_sw = s_b.reshape(nchunks[h], CPB, 128, WIN).transpose(0, 2, 1, 3)
            s_sw = np.ascontiguousarray(s_sw).reshape(nchunks[h], 128, CPB * WIN)
            idx_arrs.append(idx_sb)
            s_arrs.append(s_sw)
        core_data.append(dict(idxA=idx_arrs[0], idxB=idx_arrs[1],
                              sA=s_arrs[0], sB=s_arrs[1]))
    return meta, core_data


# ---------------- program builder ----------------------------------------------

def build_program(meta):
    import contextlib
    import concourse.bacc as bacc
    import concourse.tile as tile
    import concourse.mybir as mybir

    c = meta["cfg"]
    N, D, DOUT, C = c["N"], c["D"], c["DOUT"], c["C"]
    NSH, HALF, NT, NTPAD = c["NSH"], c["HALF"], c["NT"], c["NTPAD"]
    WIN, NWIN, KT, EPS, FH = c["WIN"], c["NWIN"], c["KT"], c["EPS"], c["FH"]
    CPB = c["GCH_BLOCKS"]
    nblocks_g = meta["nblocks_g"]
    bstart_g = meta["bstart_g"]
    nchunks = meta["nchunks"]
    nblk_pad = meta["nblk_pad"]
    bf = mybir.dt.bfloat16
    f32 = mybir.dt.float32
    RG = [list(range(C))]

    nc = bacc.Bacc("TRN2", target_bir_lowering=False, debug=False,
                   enable_asserts=False, num_devices=C)

    x_in = nc.dram_tensor("x_bf", [NTPAD, D], bf, kind="ExternalInput").ap()
    idx_t = [nc.dram_tensor(f"idx{h}", [128, nblk_pad[h] * 8], mybir.dt.int16,
                            kind="ExternalInput").ap() for h in range(2)]
    s_t = [nc.dram_tensor(f"s{h}", [nchunks[h], 128, CPB * WIN], bf,
                          kind="ExternalInput").ap() for h in range(2)]
    W_t = [nc.dram_tensor(f"W{l}", [D, D if l < 3 else DOUT], bf,
                          kind="ExternalInput").ap() for l in range(4)]
    b_t = [nc.dram_tensor(f"b{l}", [1, D if l < 3 else DOUT], bf,
                          kind="ExternalInput").ap() for l in range(4)]
    out_t = nc.dram_tensor("out", [NSH, DOUT], f32, kind="ExternalOutput").ap()

    with tile.TileContext(nc) as tc:
        with contextlib.ExitStack() as ctx:
            sb = ctx.enter_context(tc.tile_pool(name="sb", bufs=1))
            sb_h = [ctx.enter_context(tc.tile_pool(name=f"h{h}{f}", bufs=2))
                    for h in range(2) for f in range(FH)]
            sb_s = [ctx.enter_context(tc.tile_pool(name=f"ss{h}", bufs=2))
                    for h in range(2)]
            sb_hsb = ctx.enter_context(tc.tile_pool(name="hsb", bufs=3))
            sb_sq = ctx.enter_context(tc.tile_pool(name="hsq", bufs=3))
            sb_xt = ctx.enter_context(tc.tile_pool(name="xt", bufs=3))
            sb_ap = ctx.enter_context(tc.tile_pool(name="ap", bufs=3))
            ps_seg = ctx.enter_context(tc.tile_pool(name="pseg", bufs=2, space="PSUM"))
            ps_stat = ctx.enter_context(tc.tile_pool(name="pstat", bufs=1, space="PSUM"))
            ps_dense = ctx.enter_context(tc.tile_pool(name="pdense", bufs=2, space="PSUM"))
            ps_rep = ctx.enter_context(tc.tile_pool(name="prep", bufs=1, space="PSUM"))
            dram = ctx.enter_context(tc.tile_pool(name="dram", bufs=2, space="DRAM"))
            dram_tbl = ctx.enter_context(tc.tile_pool(name="dtbl", bufs=2, space="DRAM"))

            # resident constants
            ones_bf = sb.tile([128, 1], bf, tag="ones_bf")
            nc.gpsimd.memset(ones_bf[:], 1.0)
            ones1_bf = sb.tile([1, 128], bf, tag="ones1_bf")
            nc.gpsimd.memset(ones1_bf[:], 1.0)
            ones1_f = sb.tile([1, 128], f32, tag="ones1_f")
            nc.gpsimd.memset(ones1_f[:], 1.0)
            idx_sb = []
            for h in range(2):
                t = sb.tile([128, nblk_pad[h] * 8], mybir.dt.int16, tag=f"idx{h}")
                nc.sync.dma_start(t[:], idx_t[h][:])
                idx_sb.append(t)
            W_sb, b_sb = [], []
            for l in range(4):
                od = D if l < 3 else DOUT
                ks = []
                for k in range(KT):
                    t = sb.tile([128, od], bf, tag=f"W{l}k{k}")
                    nc.sync.dma_start(t[:], W_t[l][128 * k:128 * (k + 1), :])
                    ks.append(t)
                W_sb.append(ks)
                t = sb.tile([1, od], bf, tag=f"b{l}")
                nc.sync.dma_start(t[:], b_t[l][:])
                b_sb.append(t)

            def dense_layer(l, src_x):
                od = D if l < 3 else DOUT
                nfh = FH if l < 3 else 1
                fd = od // nfh
                agin = [dram.tile([NSH, fd], bf, tag=f"agin{f}", name=f"agin{f}") for f in range(nfh)]
                for t in range(NT):
                    ps = ps_dense.tile([128, od], f32, space="PSUM", tag="pdense")
                    nc.tensor.matmul(out=ps[:], lhsT=ones1_bf[:], rhs=b_sb[l][:],
                                     start=True, stop=False)
                    for k in range(KT):
                        xT = sb_xt.tile([128, 128], bf, tag="xT")
                        nc.sync.dma_start(
                            xT[:], src_x[128 * t:128 * (t + 1), 128 * k:128 * (k + 1)],
                            transpose=True)
                        nc.tensor.matmul(out=ps[:], lhsT=xT[:], rhs=W_sb[l][k][:],
                                         start=False, stop=(k == KT - 1))
                    h2 = sb_ap.tile([128, od], bf, tag="h2")
                    nc.vector.tensor_copy(h2[:], ps[:])
                    rows = min(128, NSH - 128 * t)
                    for f in range(nfh):
                        nc.sync.dma_start(agin[f][128 * t:128 * t + rows, :],
                                          h2[:rows, fd * f:fd * (f + 1)])
                return agin

            def allgather(agin, fd):
                tables = []
                for f in range(len(agin)):
                    tbl = dram_tbl.tile([N, fd], bf, tag=f"tbl{f}", name=f"tbl{f}", addr_space="Shared")
                    nc.gpsimd.collective_compute(
                        "AllGather", mybir.AluOpType.bypass,
                        replica_groups=RG,
                        ins=[agin[f].opt()],
                        outs=[tbl.opt()],
                    )
                    tables.append(tbl)
                return tables

            def segsum(l, tables):
                od = D if l < 3 else DOUT
                nfh = len(tables)
                fd = od // nfh
                # 2-deep chunk cache per stream: ch -> (s_tile, [g_tiles per f])
                cache = [dict(), dict()]

                if l < 3:
                    h_dram = dram.tile([NTPAD, D], bf, tag="h_dram")
                    col_ps = ps_stat.tile([1, D], f32, space="PSUM", tag="colps")
                    ss_ps = ps_stat.tile([1, D], f32, space="PSUM", tag="ssps")
                else:
                    h_dram = col_ps = ss_ps = None

                def get_chunk(h, ch):
                    if ch not in cache[h]:
                        while len(cache[h]) >= 2:
                            cache[h].pop(min(cache[h]))
                        st = sb_s[h].tile([128, CPB * WIN], bf, tag=f"stile{h}",
                                          name=f"stile{h}")
                        nc.sync.dma_start(st[:, :], s_t[h][ch])
                        ni = CPB * 128
                        gs = []
                        for f in range(nfh):
                            g = sb_h[h * FH + f].tile([128, CPB, fd], bf,
                                                      tag=f"g{h}{f}",
                                                      name=f"g{h}{f}")
                            nc.gpsimd.dma_gather(
                                out_ap=g[:],
                                in_ap=tables[f][HALF * h:HALF * (h + 1)],
                                idxs_ap=idx_sb[h][:, ch * CPB * 8:(ch + 1) * CPB * 8],
                                num_idxs=ni,
                                num_idxs_reg=ni,
                                elem_size=fd,
                                single_packet=False,
                            )
                            gs.append(g)
                        cache[h][ch] = (st, gs)
                    return cache[h][ch]

                for t in range(NT):
                    ps = ps_seg.tile([128, od], f32, space="PSUM", tag="pseg")
                    for win in range(NWIN):
                        blocks = []
                        for h in range(2):
                            g = (t * NWIN + win) * 2 + h
                            b0, nb = bstart_g[g], nblocks_g[g]
                            blocks += [(h, b0 + i) for i in range(nb)]
                        nb_tot = len(blocks)
                        # prefetch chunks covering this window's blocks
                        for h, b in blocks:
                            get_chunk(h, b // CPB)
                        for f in range(nfh):
                            for i, (h, b) in enumerate(blocks):
                                ch, sl = b // CPB, b % CPB
                                st, gs = get_chunk(h, ch)
                                nc.tensor.matmul(
                                    out=ps[WIN * win:WIN * (win + 1),
                                           fd * f:fd * (f + 1)],
                                    lhsT=st[:, WIN * sl:WIN * (sl + 1)],
                                    rhs=gs[f][:, sl, :],
                                    start=(i == 0), stop=(i == nb_tot - 1),
                                    skip_group_check=True,
                                )
                    if l < 3:
                        hsb = sb_hsb.tile([128, D], bf, tag="hsb")
                        nc.vector.tensor_copy(hsb[:], ps[:])
                        nc.sync.dma_start(h_dram[128 * t:128 * (t + 1), :], hsb[:])
                        hsq = sb_sq.tile([128, D], bf, tag="hsq")
                        nc.scalar.square(hsq[:], hsb[:])
                        nc.tensor.matmul(out=col_ps[:], lhsT=ones_bf[:], rhs=hsb[:],
                                         start=(t == 0), stop=(t == NT - 1),
                                         skip_group_check=True)
                        nc.tensor.matmul(out=ss_ps[:], lhsT=ones_bf[:], rhs=hsq[:],
                                         start=(t == 0), stop=(t == NT - 1),
                                         skip_group_check=True)
                    else:
                        of = sb_ap.tile([128, DOUT], f32, tag="outf")
                        nc.scalar.copy(of[:], ps[:])
                        rows = min(128, NSH - 128 * t)
                        nc.sync.dma_start(out_t[128 * t:128 * t + rows, :],
                                          of[:rows, :])
                return h_dram, col_ps, ss_ps

            def apply_layer(l, h_dram, col_ps, ss_ps, xold_dram):
                stat = sb.tile([1, 2 * D], f32, tag="stat")
                nc.vector.tensor_copy(stat[:, 0:D], col_ps[:])
                nc.vector.tensor_copy(stat[:, D:2 * D], ss_ps[:])
                ar_in = dram.tile([1, 2 * D], f32, tag="arin")
                ar_out = dram.tile([1, 2 * D], f32, tag="arout")
                nc.sync.dma_start(ar_in[:], stat[:])
                nc.gpsimd.collective_compute(
                    "AllReduce", mybir.AluOpType.add, replica_groups=RG,
                    ins=[ar_in.opt()], outs=[ar_out.opt()])
                ar = sb.tile([1, 2 * D], f32, tag="ar")
                nc.sync.dma_start(ar[:], ar_out[:])
                mu_neg = sb.tile([1, D], f32, tag="mu_neg")
                nc.vector.tensor_scalar_mul(mu_neg[:], ar[:, 0:D], -1.0 / N)
                mu_bf = sb.tile([1, D], bf, tag="mu_bf")
                nc.vector.tensor_copy(mu_bf[:], mu_neg[:])
                ss_tot = sb.tile([1, 1], f32, tag="ss_tot")
                nc.vector.tensor_reduce(ss_tot[:], ar[:, D:2 * D],
                                        axis=mybir.AxisListType.X,
                                        op=mybir.AluOpType.add)
                musq = sb.tile([1, D], f32, tag="musq")
                nc.vector.tensor_tensor(out=musq[:], in0=mu_neg[:], in1=mu_neg[:],
                                        op=mybir.AluOpType.mult)
                musq_s = sb.tile([1, 1], f32, tag="musq_s")
                nc.vector.tensor_reduce(musq_s[:], musq[:],
                                        axis=mybir.AxisListType.X,
                                        op=mybir.AluOpType.add)
                r2 = sb.tile([1, 1], f32, tag="r2")
                nc.vector.tensor_scalar(out=r2[:], in0=ss_tot[:], scalar1=1.0 / N,
                                        scalar2=None, op0=mybir.AluOpType.mult)
                nc.vector.tensor_tensor(out=r2[:], in0=r2[:], in1=musq_s[:],
                                        op=mybir.AluOpType.subtract)
                nc.vector.tensor_scalar_add(r2[:], r2[:], EPS)
                r1 = sb.tile([1, 1], f32, tag="r1")
                nc.scalar.sqrt(r1[:], r2[:])
                rs = sb.tile([1, 1], f32, tag="rs")
                nc.vector.reciprocal(rs[:], r1[:])
                mu_ps = ps_rep.tile([128, D], f32, space="PSUM", tag="repps")
                nc.tensor.matmul(out=mu_ps[:], lhsT=ones1_bf[:], rhs=mu_bf[:],
                                 start=True, stop=True)
                mu_rep = sb.tile([128, D], bf, tag="mu_rep")
                nc.vector.tensor_copy(mu_rep[:], mu_ps[:])
                rs_ps = ps_rep.tile([128, 1], f32, space="PSUM", tag="repps")
                nc.tensor.matmul(out=rs_ps[:], lhsT=ones1_f[:], rhs=rs[:],
                                 start=True, stop=True)
                rs_rep = sb.tile([128, 1], f32, tag="rs_rep")
                nc.vector.tensor_copy(rs_rep[:], rs_ps[:])

                x_dram = dram.tile([NTPAD, D], bf, tag="x_dram")
                for t in range(NT):
                    h_t = sb_ap.tile([128, D], bf, tag="h_t")
                    nc.sync.dma_start(h_t[:], h_dram[128 * t:128 * (t + 1), :])
                    t1 = sb_ap.tile([128, D], bf, tag="t1")
                    nc.vector.tensor_tensor(out=t1[:], in0=h_t[:], in1=mu_rep[:],
                                            op=mybir.AluOpType.add)
                    x1 = sb_ap.tile([128, D], bf, tag="x1")
                    nc.scalar.activation(x1[:], t1[:],
                                         mybir.ActivationFunctionType.Relu,
                                         scale=rs_rep[:, 0:1])
                    if xold_dram is not None:
                        xo = sb_ap.tile([128, D], bf, tag="xo")
                        nc.sync.dma_start(xo[:], xold_dram[128 * t:128 * (t + 1), :])
                        x1b = sb_ap.tile([128, D], bf, tag="x1b")
                        nc.vector.tensor_tensor(out=x1b[:], in0=x1[:], in1=xo[:],
                                                op=mybir.AluOpType.add)
                    else:
                        x1b = x1
                    nc.sync.dma_start(x_dram[128 * t:128 * (t + 1), :], x1b[:])
                return x_dram

            # main pipeline: x_old == x in eval mode (both equal layer output)
            x_cur = x_in
            first = True
            for l in range(3):
                agin = dense_layer(l, x_cur)
                tables = allgather(agin, D // FH)
                h_dram, col_ps, ss_ps = segsum(l, tables)
                x_cur = apply_layer(l, h_dram, col_ps, ss_ps,
                                    None if first else x_cur)
                first = False
            agin = dense_layer(3, x_cur)
            tables = allgather(agin, DOUT)
            segsum(3, tables)

    nc.compile()
    return nc


# ---------------- top-level ------------------------------------------------------

_CACHE = {}


def kernel(x, edge_w, W1, b1, W2, b2, W3, b3, Wout, bout, edge_src, edge_dst,
           _trace=False, _tmpdir=None):
    from concourse import bass_utils

    cfg = CFG
    c = _derived(cfg)
    N, D, DOUT, C, NSH, NTPAD = c["N"], c["D"], c["DOUT"], c["C"], c["NSH"], c["NTPAD"]

    if "prog" not in _CACHE:
        meta, core_data = preprocess(np.asarray(edge_src), np.asarray(edge_dst),
                                     np.asarray(edge_w), cfg)
        nc = build_program(meta)
        _CACHE["prog"] = (nc, meta, core_data)
    nc, meta, core_data = _CACHE["prog"]

    x = np.asarray(x, np.float32)
    Ws = [np.asarray(W, np.float32).astype(BF16) for W in (W1, W2, W3, Wout)]
    bs = [np.asarray(b, np.float32).astype(BF16).reshape(1, -1)
          for b in (b1, b2, b3, bout)]

    in_maps = []
    for cc in range(C):
        xs = np.zeros((NTPAD, D), BF16)
        xs[:NSH] = x[cc * NSH:(cc + 1) * NSH].astype(BF16)
        cd = core_data[cc]
        m = {"x_bf": xs, "idx0": cd["idxA"], "idx1": cd["idxB"],
             "s0": cd["sA"], "s1": cd["sB"]}
        for l in range(4):
            m[f"W{l}"] = np.ascontiguousarray(Ws[l])
            m[f"b{l}"] = np.ascontiguousarray(bs[l])
        in_maps.append(m)

    res = bass_utils.run_bass_kernel_spmd(
        nc, in_maps, core_ids=list(range(C)), trace=_trace, tmpdir=_tmpdir)
    out = np.concatenate([res.results[cc]["out"] for cc in range(C)], axis=0)
    kernel.last_exec_time_ns = res.exec_time_ns
    return out


kernel.last_exec_time_ns = None
